# revision 1
# baseline (speedup 1.0000x reference)
"""Trainium2 Bass kernel for windowed attention with relative-position bias.

Problem (hardcoded): x [32, 256, 25, 25] f32, w_qkv [256, 768], rel_emb [2401, 8],
w_out [256, 256], rel_idx [625, 625] int32. 8 heads of dim 32, n = 625 tokens.

Sharding: data-parallel over batch; 4 batches per core on 8 NeuronCores; weights
and bias replicated. No collectives.

Per-core dataflow (all matmuls bf16, f32 PSUM accumulate):
  qkv^T = w_qkv^T @ x          -> qT,kT tiles [32h..., 625]  (q pre-scaled on host)
  v     = x^T @ w_v            -> v tiles [125, 8, 33] with a ones column per head
  sim^T = k_h^T q_h + bias^T   (bias added by identity matmul from SBUF-resident
                                host-expanded bias tiles; 4-head row-packed QK)
  expsim = exp(sim^T)          (ScalarE, PSUM -> SBUF bf16; no max subtraction --
                                logits are O(10) so exp is safe in f32)
  av^T   = [v|1]^T @ expsim    (row+col packed; ones column yields softmax denom)
  out^T  = (av^T rows / denom) -> proj with w_out -> [256, 625] per batch -> HBM
"""

import sys

if "/opt/trn_rl_repo" not in sys.path:
    sys.path.insert(0, "/opt/trn_rl_repo")

import numpy as np
import ml_dtypes

B, D, WS = 32, 256, 25
N = WS * WS            # 625
H, DH = 8, 32
NC = 8                 # cores
BL = B // NC           # 4 batches per core
SCALE = DH ** -0.5
JT = 5                 # j tiles of 125
JP = N // JT           # 125
CHUNKS = ((0, 512), (512, 113))   # i chunks, bank-aligned (matmul must not cross a 512-f32 PSUM bank)

_cache = {}


def _build():
    import concourse.bass as bass
    from concourse import bacc, mybir
    from concourse.tile import TileContext
    from concourse.masks import make_identity

    f32 = mybir.dt.float32
    bf16 = mybir.dt.bfloat16

    nc = bacc.Bacc()
    x_ext = nc.declare_dram_parameter("x", [BL, D, N], bf16, isOutput=False)
    wqkv_ext = nc.declare_dram_parameter("wqkv", [D, 3 * D], bf16, isOutput=False)
    wout_ext = nc.declare_dram_parameter("wout", [D, D], bf16, isOutput=False)
    biasT_ext = nc.declare_dram_parameter("biasT", [H, JT, JP, N], bf16, isOutput=False)
    out_ext = nc.declare_dram_parameter("out", [BL, D, N], f32, isOutput=True)

    with TileContext(nc) as tc:
        with (
            tc.tile_pool(name="const", bufs=1) as const,
            tc.tile_pool(name="xp", bufs=2) as xp,
            tc.tile_pool(name="qk", bufs=2) as qkp,
            tc.tile_pool(name="vp", bufs=2) as vp,
            tc.tile_pool(name="es", bufs=3) as esp,
            tc.tile_pool(name="div", bufs=2) as divp,
            tc.tile_pool(name="ot", bufs=2) as otp,
            tc.tile_pool(name="res", bufs=2) as resp,
            tc.tile_pool(name="sim", bufs=2, space="PSUM") as simp,
            tc.tile_pool(name="acc", bufs=2, space="PSUM") as accp,
        ):
            wqkv_sb = const.tile([128, 2, 3 * D], bf16)
            nc.sync.dma_start(out=wqkv_sb, in_=wqkv_ext.rearrange("(k p) c -> p k c", p=128))
            wout_sb = const.tile([128, 2, D], bf16)
            nc.sync.dma_start(out=wout_sb, in_=wout_ext.rearrange("(k p) c -> p k c", p=128))
            biasT_sb = const.tile([JP, H, JT, N], bf16)
            for h in range(H):
                for jt in range(JT):
                    nc.sync.dma_start(out=biasT_sb[:, h, jt, :], in_=biasT_ext[h, jt])
            ident = const.tile([128, 128], bf16)
            make_identity(nc, ident)

            for b in range(BL):
                x_sb = xp.tile([128, 2, N], bf16)
                nc.sync.dma_start(out=x_sb, in_=x_ext[b].rearrange("(k p) n -> p k n", p=128))

                # q^T, k^T tiles: qkT_sb[:, m, :], m in 0..3 (q: 0-1, k: 2-3)
                qkT_sb = qkp.tile([128, 4, N], bf16)
                for m in range(4):
                    ps = accp.tile([128, N], f32, tag="acc")
                    for kt in range(2):
                        for lo, sz in CHUNKS:
                            nc.tensor.matmul(
                                ps[:, lo:lo + sz],
                                wqkv_sb[:, kt, m * 128:(m + 1) * 128],
                                x_sb[:, kt, lo:lo + sz],
                                start=(kt == 0), stop=(kt == 1))
                    nc.scalar.copy(qkT_sb[:, m, :], ps)

                # v in [j, head, dh|1] layout with ones column per head
                v_sb = vp.tile([JP, JT, H, DH + 1], bf16)
                for nt in range(JT):
                    psv = accp.tile([JP, 2 * DH * H], f32, tag="acc")
                    for kt in range(2):
                        nc.tensor.matmul(
                            psv[:, :D],
                            x_sb[:, kt, nt * JP:(nt + 1) * JP],
                            wqkv_sb[:, kt, 2 * D:3 * D],
                            start=(kt == 0), stop=(kt == 1))
                    nc.scalar.copy(
                        v_sb[:, nt, :, 0:DH],
                        psv[:, :D].rearrange("p (h d) -> p h d", h=H))
                nc.gpsimd.memset(v_sb[:, :, :, DH:DH + 1], 1.0)

                outT_sb = otp.tile([128, 2, N], bf16)
                # software-pipelined attention: issue av(k-1) after QK(k) so the
                # PE never stalls in-order on es(k) being produced by ACT/DVE
                iters = [(2 * g + h2, jt) for g in range(H // 2)
                         for h2 in range(2) for jt in range(JT)]
                av_tiles = {}
                pending = []

                def issue_av(h, jt, es):
                    ro = 64 * (h % 2)
                    if jt == 0 and h % 2 == 0:
                        av_t = accp.tile([128, 2, 512], f32, tag="acc")
                        av_tiles[h // 2] = av_t
                    av = av_tiles[h // 2]
                    for ci, (lo, sz) in enumerate(CHUNKS):
                        nc.tensor.matmul(
                            av[ro:ro + DH + 1, ci, 0:sz],
                            v_sb[0:JP, jt, h, :],
                            es[0:JP, lo:lo + sz],
                            start=(jt == 0), stop=(jt == JT - 1),
                            tile_position=(0, ro))

                def issue_div(h):
                    ro = 64 * (h % 2)
                    hq, mt = (h % 4) * 32, h // 4
                    av = av_tiles[h // 2]
                    rcp = divp.tile([1, 2, 512], f32, tag="rcp")
                    rb = divp.tile([128, 2, 512], f32, tag="rb")
                    for ci, (lo, sz) in enumerate(CHUNKS):
                        nc.vector.reciprocal(
                            rcp[:, ci, 0:sz], av[ro + DH:ro + DH + 1, ci, 0:sz])
                    nc.gpsimd.partition_broadcast(rb, rcp, channels=128)
                    for ci, (lo, sz) in enumerate(CHUNKS):
                        nc.vector.tensor_mul(outT_sb[hq:hq + 32, mt, lo:lo + sz],
                                             av[ro:ro + DH, ci, 0:sz],
                                             rb[ro:ro + DH, ci, 0:sz])

                for k, (h, jt) in enumerate(iters + [(None, None)]):
                    if h is not None:
                        hq, mt = (h % 4) * 32, h // 4
                        sim = simp.tile([JP, N], f32, tag="sim")
                        for ci, (lo, sz) in enumerate(CHUNKS):
                            nc.tensor.matmul(
                                sim[:, lo:lo + sz],
                                qkT_sb[hq:hq + 32, 2 + mt, jt * JP:(jt + 1) * JP],
                                qkT_sb[hq:hq + 32, mt, lo:lo + sz],
                                start=True, stop=True, tile_position=(hq, 0))
                        esr = esp.tile([JP, N], bf16, tag="esr")
                        nc.scalar.activation(out=esr, in_=sim,
                                             func=mybir.ActivationFunctionType.Exp)
                        es = esp.tile([JP, N], bf16, tag="es")
                        nc.vector.tensor_mul(es, esr, biasT_sb[0:JP, h, jt, :])
                        pending.append((h, jt, es))
                    if len(pending) > (1 if h is not None else 0):
                        ph, pjt, pes = pending.pop(0)
                        issue_av(ph, pjt, pes)
                        if pjt == JT - 1:
                            issue_div(ph)
                while pending:
                    ph, pjt, pes = pending.pop(0)
                    issue_av(ph, pjt, pes)
                    if pjt == JT - 1:
                        issue_div(ph)

                # output projection: out^T[c, i] = sum_d wout[d, c] outT[d, i]
                for ct in range(2):
                    psp = accp.tile([128, N], f32, tag="acc")
                    for kt in range(2):
                        for lo, sz in CHUNKS:
                            nc.tensor.matmul(
                                psp[:, lo:lo + sz],
                                wout_sb[:, kt, ct * 128:(ct + 1) * 128],
                                outT_sb[:, kt, lo:lo + sz],
                                start=(kt == 0), stop=(kt == 1))
                    o_t = resp.tile([128, N], f32)
                    nc.vector.tensor_copy(o_t, psp)
                    nc.sync.dma_start(out=out_ext[b, ct * 128:(ct + 1) * 128, :], in_=o_t)

    nc.compile()
    return nc


def _get_nc():
    if "nc" not in _cache:
        _cache["nc"] = _build()
    return _cache["nc"]


def kernel(x, w_qkv, rel_emb, w_out, rel_idx):
    from concourse.bass_utils import run_bass_kernel_spmd

    nc = _get_nc()
    bf = ml_dtypes.bfloat16

    wqkv_s = np.array(w_qkv, dtype=np.float32, copy=True)
    wqkv_s[:, :D] *= SCALE                      # fold q scaling into weights
    wqkv_b = wqkv_s.astype(bf)
    wout_b = np.asarray(w_out, dtype=np.float32).astype(bf)
    # bias[h, i, j] = rel_emb[rel_idx[i, j], h];  biasT[h, j, i] = bias[h, i, j]
    bias = np.asarray(rel_emb, dtype=np.float32)[np.asarray(rel_idx)]   # [i, j, h]
    biasT = np.exp(np.ascontiguousarray(bias.transpose(2, 1, 0))).reshape(H, JT, JP, N).astype(bf)

    xf = np.asarray(x, dtype=np.float32).reshape(B, D, N).astype(bf)
    in_maps = [
        {"x": xf[c * BL:(c + 1) * BL], "wqkv": wqkv_b, "wout": wout_b, "biasT": biasT}
        for c in range(NC)
    ]
    res = run_bass_kernel_spmd(nc, in_maps, list(range(NC)))
    out = np.concatenate([res.results[c]["out"] for c in range(NC)], axis=0)
    return out.reshape(B, D, WS, WS).astype(np.float32)

